# revision 32
# baseline (speedup 1.0000x reference)
"""Trainium2 Bass kernel for nn_DownsamplePoly (resample_poly up=5/down=64,
1345-tap filter, x:[16,1280000,4] fp32 -> y:[16,100000,4] fp32).

Strategy (v4)
-------------
Math: y[n] = sum_i h[64n + 1344 - 5i] x_pad[i]  (x_pad[i] = x[i-128]).
J-tiles of MT=120 outputs advance exactly 12 aligned 128-sample chunks; the
banded weights W_j[k, m] = h[64m+1344-640j-5k] are tile-independent and each
chunk j only touches a <=30-wide window of the 120 outputs (the band slides
10 outputs/chunk). The 14 chunk-matmuls per J-tile are therefore emitted as
17 narrow column-tiles (64/32 wide) on disjoint 32-aligned PE column groups
(tile_position), which the PE array runs CONCURRENTLY via separate XBUSes:
6 rounds of ~504 cycles instead of 14 serial matmuls (~2x).

Wire format: x is quantized host-side to a uniform grid of step s with
second-order noise shaping (round a double-cumsum, double-diff): integers in
[-16,16], exact in fp8e4m3, quantization noise pushed out of the passband
(end-to-end rel err ~3e-3 vs gate 2e-2). Weights are fp16 pre-scaled by s;
the PE runs mixed fp16(stationary) x fp8(moving) matmuls at full rate with
half the HBM bytes. ncol packs 8 (batch,chan) pairs x up-to-63 J-tiles=504.

DMA: input slabs on sync queue, weights+outputs on scalar queue; supertile
sizes ramp [4,16,48,63...,10] so compute starts early and drains fast. A
junk-matmul warmup pump trips the HAM clock gate to full rate during the
initial DMA wait. 8 cores split the batch dim (2 batches/core).
"""

import os
from contextlib import ExitStack

import numpy as np
import ml_dtypes

# ---- geometry (hardcoded for this problem) ----
B, T, C = 16, 1_280_000, 4
N_OUT = 100_000
SU, DU = 50, 640          # -> up=5, down=64
MT = 120                  # outputs per J-tile (psum partition dim)
ADV = 12                  # chunk advance per J-tile (12*128 = 120*64/5)
KCH = 14                  # nonzero chunk-matmuls per J-tile
JPS = [4, 16, 48] + [63] * 11 + [45, 18, 6, 4]   # J-tiles per supertile; sum = 834
NS = len(JPS)
JTOT = sum(JPS)           # 834 (>= ceil(100000/120))
QTOT_PAD = 12 * (JTOT - JPS[-1]) + 12 * (JPS[-1] + 1)
PAD_L = 128               # x_pad[i] = x[i-128]
BPC = B // 8              # batches per core = 2
NBC = BPC * C             # 8 (b,c) pairs per core
# slab layout per supertile: chunk (r, q12) at block r*jp+min(r,2)+q12;
# only r<2 needs the extra q12=jp block (read by chunks j=12,13)
CTOT = 8 * (12 * JTOT + 2 * NS)   # total xt columns

# col-tile schedule: (chunk j, psum col_lo, width, start_flag), 6 rounds.
# chunk m-windows: j0[0,9] j1[0,19] j2[0,29] j3[10,39] j4[20,49] j5[30,59]
# j6[40,69] j7[50,79] j8[60,89] j9[70,99] j10[80,109] j11[90,119]
# j12[100,119] j13[110,119]
TILES = [
    (3, 0, 64, True), (9, 64, 56, True),
    (4, 0, 64, False), (10, 64, 56, False),
    (5, 0, 64, False), (11, 64, 56, False),
    (0, 0, 32, False), (6, 32, 32, False), (7, 64, 32, False), (12, 96, 24, False),
    (1, 0, 32, False), (7, 32, 32, False), (8, 64, 32, False), (13, 96, 24, False),
    (2, 0, 32, False), (8, 32, 32, False), (6, 64, 32, False),
]
WOFFS = np.cumsum([0] + [t[2] for t in TILES]).tolist()
WTOT = WOFFS[-1]

_NC_CACHE = {}


def build_weights(h):
    """W[j, k, m] = h_ext[64m + 1344 - 640j - 5k] for j in [0, KCH)."""
    h_ext = np.zeros(1345 + 64 * MT, dtype=np.float64)
    h_ext[: h.shape[0]] = h
    j = np.arange(KCH)[:, None, None]
    k = np.arange(128)[None, :, None]
    m = np.arange(MT)[None, None, :]
    idx = 64 * m + 1344 - 640 * j - 5 * k
    valid = (idx >= 0) & (idx <= 1344)
    W = np.where(valid, h_ext[np.clip(idx, 0, 1344)], 0.0)
    # sanity: the col-tile schedule must cover every nonzero weight column
    for jj in range(KCH):
        nz = np.where(np.any(W[jj] != 0, axis=0))[0]
        cov = np.zeros(MT, dtype=bool)
        for (tj, lo, w, _s) in TILES:
            if tj == jj:
                cov[lo:lo + w] = True
        assert cov[nz].all(), f"chunk {jj} window {nz.min()}..{nz.max()} uncovered"
    return W


def _build_nc():
    import concourse.bacc as bacc
    import concourse.tile as tile
    import concourse.mybir as mybir

    F32 = mybir.dt.float32
    F16 = mybir.dt.float16
    F8 = mybir.dt.float8e4

    TAILC = 8 * sum(JPS[NS - 4:])   # tail supertiles' merged output cols
    nc = bacc.Bacc()
    xt = nc.dram_tensor("xt", [128, CTOT], F8, kind="ExternalInput")
    w = nc.dram_tensor("w", [128, WTOT], F16, kind="ExternalInput")
    y = nc.dram_tensor("y", [NS, MT, 504], F16, kind="ExternalOutput")
    y2 = nc.dram_tensor("y2", [MT, TAILC], F16, kind="ExternalOutput")

    with tile.TileContext(nc) as tc, ExitStack() as ctx:
        const = ctx.enter_context(tc.tile_pool(name="const", bufs=1))
        junk = ctx.enter_context(tc.tile_pool(name="junk", bufs=1))
        wt = const.tile([128, WTOT], F16)
        nc.scalar.dma_start(wt[:], w[:, :])

        slabs = ctx.enter_context(tc.tile_pool(name="slabs", bufs=10))
        tpool = ctx.enter_context(tc.tile_pool(name="tp", bufs=1))
        psum = ctx.enter_context(tc.tile_pool(name="ps", bufs=4, space="PSUM"))
        wpsum = ctx.enter_context(tc.tile_pool(name="wps", bufs=1, space="PSUM"))
        spool = ctx.enter_context(tc.tile_pool(name="sp", bufs=4))

        # HAM warmup: ~9 junk matmuls (~3.5us cold) trip the PE clock gate
        # to full rate while the first slab DMAs are still in flight.
        jt = junk.tile([128, 504], F16)
        nc.vector.memset(jt[:], 0)
        wps = wpsum.tile([120, 504], F32, tag="wps")

        def warm(n):
            for _ in range(n):
                nc.tensor.matmul(
                    wps[:], jt[:, :120], jt[:, :504],
                    start=True, stop=True, skip_group_check=True,
                )

        # 6 junk MMs before ST0, more interleaved between ramp STs: keeps
        # the PE busy through the first DMA waits and trips the HAM clock
        # gate (~3.4us sustained) without serializing ahead of real work
        warm(6)
        WARM_AFTER = {0: 2, 1: 2, 2: 1}

        st2 = tpool.tile([MT, TAILC], F16)
        toff = 0
        off = 0
        for s, jp in enumerate(JPS):
            ncol = 8 * jp
            L = 8 * (12 * jp + 2)
            slab = slabs.tile([128, 8 * (12 * 63 + 2)], F8, tag="slab")
            nc.sync.dma_start(slab[:, :L], xt[:, off:off + L])
            ps = psum.tile([MT, 504], F32, tag="ps")
            for ti, (j, lo, wd_, st) in enumerate(TILES):
                a, r = divmod(j, ADV)
                base = (r * jp + min(r, 2) + a) * 8
                nc.tensor.matmul(
                    ps[lo:lo + wd_, :ncol],
                    wt[:, WOFFS[ti]:WOFFS[ti] + wd_],
                    slab[:, base:base + ncol],
                    start=st, stop=(ti == len(TILES) - 1),
                    skip_group_check=True,
                    tile_position=(0, lo),
                )
            if s >= NS - 4:
                # tail outputs accumulate in one staging tile; a single
                # DMA on the (by-then idle) sync queue flushes them all
                nc.vector.tensor_copy(st2[:, toff:toff + ncol], ps[:, :ncol])
                toff += ncol
                if s == NS - 1:
                    nc.sync.dma_start(y2[:, :], st2[:])
            else:
                st_ = spool.tile([MT, 504], F16, tag="st")
                nc.vector.tensor_copy(st_[:, :ncol], ps[:, :ncol])
                nc.scalar.dma_start(y[s, :, :ncol], st_[:, :ncol])
            warm(WARM_AFTER.get(s, 0))
            off += L
    nc.compile()
    return nc


def _quantize_shaped(xc, step):
    """2nd-order noise-shaped quantization to integer grid (fp8-exact)."""
    s2 = np.cumsum(np.cumsum(xc.astype(np.float64) / step, axis=1), axis=1)
    Q = np.rint(s2)
    pre = np.zeros((xc.shape[0], 2, xc.shape[2]))
    d = np.diff(np.concatenate([pre, Q], axis=1), n=2, axis=1)
    assert np.abs(d).max() <= 16, np.abs(d).max()
    return d.astype(ml_dtypes.float8_e4m3)


def kernel(x, h, su, du):
    assert int(su) == SU and int(du) == DU
    from concourse.bass_utils import run_bass_kernel_spmd

    x = np.asarray(x)
    h = np.asarray(h, dtype=np.float64)
    assert x.shape == (B, T, C), x.shape

    W = build_weights(h)
    if "nc" not in _NC_CACHE:
        _NC_CACHE["nc"] = _build_nc()
    nc = _NC_CACHE["nc"]

    step = float(np.abs(x).max()) / 11.9
    wflat = np.concatenate(
        [(W[j][:, lo:lo + wd_] * step) for (j, lo, wd_, _s) in TILES], axis=1
    ).astype(np.float16)
    assert wflat.shape == (128, WTOT)

    # per-supertile chunk gather indices: block r*jp+min(r,2)+q12 <- chunk
    # Qs + 12*q12 + r  (r<2 carries one extra block, q12=jp)
    st_idx = []
    Qs = 0
    for jp in JPS:
        idx = []
        for r in range(ADV):
            nb = jp + (1 if r < 2 else 0)
            idx.extend(Qs + ADV * q12 + r for q12 in range(nb))
        st_idx.append(np.array(idx))
        Qs += ADV * jp

    in_maps = []
    for core in range(8):
        xs = x[core * BPC:(core + 1) * BPC]
        d8 = _quantize_shaped(xs, step)
        xp = np.zeros((BPC, QTOT_PAD * 128, C), dtype=ml_dtypes.float8_e4m3)
        xp[:, PAD_L:PAD_L + T] = d8
        xall = np.ascontiguousarray(
            xp.reshape(BPC, QTOT_PAD, 128, C).transpose(2, 1, 0, 3)
        ).reshape(128, QTOT_PAD, NBC)
        parts = [xall[:, idx, :].reshape(128, -1) for idx in st_idx]
        xtc = np.ascontiguousarray(np.concatenate(parts, axis=1))
        assert xtc.shape == (128, CTOT), xtc.shape
        in_maps.append({"xt": xtc, "w": wflat})

    trace = bool(os.environ.get("BASS_KERNEL_TRACE"))
    res = run_bass_kernel_spmd(
        nc, in_maps, core_ids=list(range(8)), trace=trace
    )
    kernel.last_results = res

    out = np.empty((B, N_OUT, C), dtype=np.float32)
    for core in range(8):
        yd = res.results[core]["y"]
        yd2 = res.results[core]["y2"]
        JB = 0
        toff = 0
        for s, jp in enumerate(JPS):
            if s >= NS - 4:
                blk = yd2[:, toff:toff + 8 * jp].reshape(MT, jp, BPC, C)
                toff += 8 * jp
            else:
                blk = yd[s, :, :8 * jp].reshape(MT, jp, BPC, C)
            blk = blk.transpose(2, 1, 0, 3).reshape(BPC, jp * MT, C)
            n0 = MT * JB
            n1 = min(n0 + jp * MT, N_OUT)
            if n1 > n0:
                out[core * BPC:(core + 1) * BPC, n0:n1] = blk[:, : n1 - n0]
            JB += jp
    return out


if __name__ == "__main__":
    rng = np.random.default_rng(0)
    x = rng.standard_normal((B, T, C)).astype(np.float32)
    import sys
    sys.path.insert(0, "/root/problem")
    from reference import _make_filter
    h = _make_filter(DU, SU, T)
    y = kernel(x, h, SU, DU)
    print("y", y.shape, y.dtype)


# revision 33
# speedup vs baseline: 1.1254x; 1.1254x over previous
"""Trainium2 Bass kernel for nn_DownsamplePoly (resample_poly up=5/down=64,
1345-tap filter, x:[16,1280000,4] fp32 -> y:[16,100000,4] fp32).

Strategy (v4)
-------------
Math: y[n] = sum_i h[64n + 1344 - 5i] x_pad[i]  (x_pad[i] = x[i-128]).
J-tiles of MT=120 outputs advance exactly 12 aligned 128-sample chunks; the
banded weights W_j[k, m] = h[64m+1344-640j-5k] are tile-independent and each
chunk j only touches a <=30-wide window of the 120 outputs (the band slides
10 outputs/chunk). The 14 chunk-matmuls per J-tile are therefore emitted as
17 narrow column-tiles (64/32 wide) on disjoint 32-aligned PE column groups
(tile_position), which the PE array runs CONCURRENTLY via separate XBUSes:
6 rounds of ~504 cycles instead of 14 serial matmuls (~2x).

Wire format: x is quantized host-side to a uniform grid of step s with
second-order noise shaping (round a double-cumsum, double-diff): integers in
[-16,16], exact in fp8e4m3, quantization noise pushed out of the passband
(end-to-end rel err ~3e-3 vs gate 2e-2). Weights are fp16 pre-scaled by s;
the PE runs mixed fp16(stationary) x fp8(moving) matmuls at full rate with
half the HBM bytes. ncol packs 8 (batch,chan) pairs x up-to-63 J-tiles=504.

DMA: input slabs on sync queue, weights+outputs on scalar queue; supertile
sizes ramp [4,16,48,63...,10] so compute starts early and drains fast. A
junk-matmul warmup pump trips the HAM clock gate to full rate during the
initial DMA wait. 8 cores split the batch dim (2 batches/core).
"""

import os
from contextlib import ExitStack

import numpy as np
import ml_dtypes

# ---- geometry (hardcoded for this problem) ----
B, T, C = 16, 1_280_000, 4
N_OUT = 100_000
SU, DU = 50, 640          # -> up=5, down=64
MT = 120                  # outputs per J-tile (psum partition dim)
ADV = 12                  # chunk advance per J-tile (12*128 = 120*64/5)
KCH = 14                  # nonzero chunk-matmuls per J-tile
JPS = [4, 16, 48] + [63] * 11 + [45, 18, 6, 4]   # J-tiles per supertile; sum = 834
NS = len(JPS)
JTOT = sum(JPS)           # 834 (>= ceil(100000/120))
QTOT_PAD = 12 * (JTOT - JPS[-1]) + 12 * (JPS[-1] + 1)
PAD_L = 128               # x_pad[i] = x[i-128]
BPC = B // 8              # batches per core = 2
NBC = BPC * C             # 8 (b,c) pairs per core
# slab layout per supertile: chunk (r, q12) at block r*jp+min(r,2)+q12;
# only r<2 needs the extra q12=jp block (read by chunks j=12,13)
CTOT = 8 * (12 * JTOT + 2 * NS)   # total xt columns

# col-tile schedule: (chunk j, psum col_lo, width, start_flag), 6 rounds.
# chunk m-windows: j0[0,9] j1[0,19] j2[0,29] j3[10,39] j4[20,49] j5[30,59]
# j6[40,69] j7[50,79] j8[60,89] j9[70,99] j10[80,109] j11[90,119]
# j12[100,119] j13[110,119]
TILES = [
    (3, 0, 64, True), (9, 64, 56, True),
    (4, 0, 64, False), (10, 64, 56, False),
    (5, 0, 64, False), (11, 64, 56, False),
    (0, 0, 32, False), (6, 32, 32, False), (7, 64, 32, False), (12, 96, 24, False),
    (1, 0, 32, False), (7, 32, 32, False), (8, 64, 32, False), (13, 96, 24, False),
    (2, 0, 32, False), (8, 32, 32, False), (6, 64, 32, False),
]
WOFFS = np.cumsum([0] + [t[2] for t in TILES]).tolist()
WTOT = WOFFS[-1]

_NC_CACHE = {}


def build_weights(h):
    """W[j, k, m] = h_ext[64m + 1344 - 640j - 5k] for j in [0, KCH)."""
    h_ext = np.zeros(1345 + 64 * MT, dtype=np.float64)
    h_ext[: h.shape[0]] = h
    j = np.arange(KCH)[:, None, None]
    k = np.arange(128)[None, :, None]
    m = np.arange(MT)[None, None, :]
    idx = 64 * m + 1344 - 640 * j - 5 * k
    valid = (idx >= 0) & (idx <= 1344)
    W = np.where(valid, h_ext[np.clip(idx, 0, 1344)], 0.0)
    # sanity: the col-tile schedule must cover every nonzero weight column
    for jj in range(KCH):
        nz = np.where(np.any(W[jj] != 0, axis=0))[0]
        cov = np.zeros(MT, dtype=bool)
        for (tj, lo, w, _s) in TILES:
            if tj == jj:
                cov[lo:lo + w] = True
        assert cov[nz].all(), f"chunk {jj} window {nz.min()}..{nz.max()} uncovered"
    return W


def _build_nc():
    import concourse.bacc as bacc
    import concourse.tile as tile
    import concourse.mybir as mybir

    F32 = mybir.dt.float32
    F16 = mybir.dt.float16
    F8 = mybir.dt.float8e4

    TAILC = 8 * sum(JPS[NS - 4:])   # tail supertiles' merged output cols
    nc = bacc.Bacc()
    xt = nc.dram_tensor("xt", [128, CTOT], F8, kind="ExternalInput")
    w = nc.dram_tensor("w", [128, WTOT], F16, kind="ExternalInput")
    y = nc.dram_tensor("y", [NS, MT, 504], F16, kind="ExternalOutput")
    y2 = nc.dram_tensor("y2", [MT, TAILC], F16, kind="ExternalOutput")

    with tile.TileContext(nc) as tc, ExitStack() as ctx:
        const = ctx.enter_context(tc.tile_pool(name="const", bufs=1))
        junk = ctx.enter_context(tc.tile_pool(name="junk", bufs=1))
        wt = const.tile([128, WTOT], F16)
        nc.scalar.dma_start(wt[:], w[:, :])

        slabs = ctx.enter_context(tc.tile_pool(name="slabs", bufs=12))
        tpool = ctx.enter_context(tc.tile_pool(name="tp", bufs=1))
        psum = ctx.enter_context(tc.tile_pool(name="ps", bufs=4, space="PSUM"))
        wpsum = ctx.enter_context(tc.tile_pool(name="wps", bufs=1, space="PSUM"))
        spool = ctx.enter_context(tc.tile_pool(name="sp", bufs=4))

        # HAM warmup: ~9 junk matmuls (~3.5us cold) trip the PE clock gate
        # to full rate while the first slab DMAs are still in flight.
        jt = junk.tile([128, 504], F16)
        nc.vector.memset(jt[:], 0)
        wps = wpsum.tile([120, 504], F32, tag="wps")

        def warm(n):
            for _ in range(n):
                nc.tensor.matmul(
                    wps[:], jt[:, :120], jt[:, :504],
                    start=True, stop=True, skip_group_check=True,
                )

        # 6 junk MMs before ST0, more interleaved between ramp STs: keeps
        # the PE busy through the first DMA waits and trips the HAM clock
        # gate (~3.4us sustained) without serializing ahead of real work
        warm(6)
        WARM_AFTER = {0: 2, 1: 2, 2: 1}

        st2 = tpool.tile([MT, TAILC], F16)
        toff = 0
        off = 0
        for s, jp in enumerate(JPS):
            ncol = 8 * jp
            L = 8 * (12 * jp + 2)
            slab = slabs.tile([128, 8 * (12 * 63 + 2)], F8, tag="slab")
            nc.sync.dma_start(slab[:, :L], xt[:, off:off + L])
            ps = psum.tile([MT, 504], F32, tag="ps")
            for ti, (j, lo, wd_, st) in enumerate(TILES):
                a, r = divmod(j, ADV)
                base = (r * jp + min(r, 2) + a) * 8
                nc.tensor.matmul(
                    ps[lo:lo + wd_, :ncol],
                    wt[:, WOFFS[ti]:WOFFS[ti] + wd_],
                    slab[:, base:base + ncol],
                    start=st, stop=(ti == len(TILES) - 1),
                    skip_group_check=True,
                    tile_position=(0, lo),
                )
            if s >= NS - 4:
                # tail outputs accumulate in one staging tile; a single
                # DMA on the (by-then idle) sync queue flushes them all
                nc.vector.tensor_copy(st2[:, toff:toff + ncol], ps[:, :ncol])
                toff += ncol
                if s == NS - 1:
                    nc.sync.dma_start(y2[:, :], st2[:])
            else:
                st_ = spool.tile([MT, 504], F16, tag="st")
                nc.vector.tensor_copy(st_[:, :ncol], ps[:, :ncol])
                nc.scalar.dma_start(y[s, :, :ncol], st_[:, :ncol])
            warm(WARM_AFTER.get(s, 0))
            off += L
    nc.compile()
    return nc


def _quantize_shaped(xc, step):
    """2nd-order noise-shaped quantization to integer grid (fp8-exact)."""
    s2 = np.cumsum(np.cumsum(xc.astype(np.float64) / step, axis=1), axis=1)
    Q = np.rint(s2)
    pre = np.zeros((xc.shape[0], 2, xc.shape[2]))
    d = np.diff(np.concatenate([pre, Q], axis=1), n=2, axis=1)
    assert np.abs(d).max() <= 16, np.abs(d).max()
    return d.astype(ml_dtypes.float8_e4m3)


def kernel(x, h, su, du):
    assert int(su) == SU and int(du) == DU
    from concourse.bass_utils import run_bass_kernel_spmd

    x = np.asarray(x)
    h = np.asarray(h, dtype=np.float64)
    assert x.shape == (B, T, C), x.shape

    W = build_weights(h)
    if "nc" not in _NC_CACHE:
        _NC_CACHE["nc"] = _build_nc()
    nc = _NC_CACHE["nc"]

    step = float(np.abs(x).max()) / 11.9
    wflat = np.concatenate(
        [(W[j][:, lo:lo + wd_] * step) for (j, lo, wd_, _s) in TILES], axis=1
    ).astype(np.float16)
    assert wflat.shape == (128, WTOT)

    # per-supertile chunk gather indices: block r*jp+min(r,2)+q12 <- chunk
    # Qs + 12*q12 + r  (r<2 carries one extra block, q12=jp)
    st_idx = []
    Qs = 0
    for jp in JPS:
        idx = []
        for r in range(ADV):
            nb = jp + (1 if r < 2 else 0)
            idx.extend(Qs + ADV * q12 + r for q12 in range(nb))
        st_idx.append(np.array(idx))
        Qs += ADV * jp

    in_maps = []
    for core in range(8):
        xs = x[core * BPC:(core + 1) * BPC]
        d8 = _quantize_shaped(xs, step)
        xp = np.zeros((BPC, QTOT_PAD * 128, C), dtype=ml_dtypes.float8_e4m3)
        xp[:, PAD_L:PAD_L + T] = d8
        xall = np.ascontiguousarray(
            xp.reshape(BPC, QTOT_PAD, 128, C).transpose(2, 1, 0, 3)
        ).reshape(128, QTOT_PAD, NBC)
        parts = [xall[:, idx, :].reshape(128, -1) for idx in st_idx]
        xtc = np.ascontiguousarray(np.concatenate(parts, axis=1))
        assert xtc.shape == (128, CTOT), xtc.shape
        in_maps.append({"xt": xtc, "w": wflat})

    trace = bool(os.environ.get("BASS_KERNEL_TRACE"))
    res = run_bass_kernel_spmd(
        nc, in_maps, core_ids=list(range(8)), trace=trace
    )
    kernel.last_results = res

    out = np.empty((B, N_OUT, C), dtype=np.float32)
    for core in range(8):
        yd = res.results[core]["y"]
        yd2 = res.results[core]["y2"]
        JB = 0
        toff = 0
        for s, jp in enumerate(JPS):
            if s >= NS - 4:
                blk = yd2[:, toff:toff + 8 * jp].reshape(MT, jp, BPC, C)
                toff += 8 * jp
            else:
                blk = yd[s, :, :8 * jp].reshape(MT, jp, BPC, C)
            blk = blk.transpose(2, 1, 0, 3).reshape(BPC, jp * MT, C)
            n0 = MT * JB
            n1 = min(n0 + jp * MT, N_OUT)
            if n1 > n0:
                out[core * BPC:(core + 1) * BPC, n0:n1] = blk[:, : n1 - n0]
            JB += jp
    return out


if __name__ == "__main__":
    rng = np.random.default_rng(0)
    x = rng.standard_normal((B, T, C)).astype(np.float32)
    import sys
    sys.path.insert(0, "/root/problem")
    from reference import _make_filter
    h = _make_filter(DU, SU, T)
    y = kernel(x, h, SU, DU)
    print("y", y.shape, y.dtype)
